# revision 31
# baseline (speedup 1.0000x reference)
"""Aaren prefix online-softmax attention on 8 TRN2 NeuronCores.

Math: s = K @ q [L]; out[t] = sum_{i<=t} exp(s_i - M_t) V_i / sum_{i<=t}
exp(s_i - M_t), with M = cummax(s). Since the quotient is invariant to the
stabilizer and M is monotone, every weight exp(s_i - M_t) lies in [0, 1]:
underflow is harmless and there is no overflow, so the whole prefix scan
becomes per-block lower-triangular matmuls.

Sharding: K by L-rows (1024/core) for the s-stage, then AllGather s (32KB);
V and out by D-columns (256/core) for the scan stage. Each core:
  stage 0: q broadcast to [128, D] via a stride-0 DMA
  stage 1: s_part = K_shard @ q (vector-engine custom fused
           multiply-reduce; the raw TENSOR_TENSOR_REDUCE opcode needs the
           generated DVE uop table, hence _custom_dve)
  collective: AllGather s -> full [8192]
  stage 2: cummax M via tensor_tensor_scan (chunk-major [128,64] + a
           [1,128] chunk-end pass bridged by two tiny PE transposes)
  stage 3: 65 blocks of 127 rows. Per block: one [2,128]x[2,rows] matmul
           builds S1[p,j] = s_ext[p] - M[t0+j] (p=0 is a carry pseudo-row
           whose weights end up being exp(M[127b-1] - M[t])), exp on the
           scalar engine, causal mask via affine_select with the columns
           rotated by one so output row 0 = the block-local tail T_b,
           then one [128,1+rows]x[128,257] matmul against [0; V_block|1].
           The global carry recursion C_{b+1} = e_b*C_b + T_b runs
           entirely on the vector engine (one scalar_tensor_tensor per
           block reading T_b straight from PSUM row 0, partition 0), and
           each block b>=1 adds its full prefix with one rank-1 matmul
           (weights = a_m row 0, rhs = C_b) accumulated into PSUM before
           the reciprocal-normalize and the output DMA.

Hardware constraints baked in: compute-engine SBUF/PSUM access patterns
must start at partition 0/32/64/96 (DMA is exempt); PE reads SBUF only;
single-partition flattened DMA sources must be expressed as 3D
[1, p, j]-destination views or the NEFF fails to load.
"""
import sys

sys.path.insert(0, "/opt/trn_rl_repo")

import numpy as np

L = 8192
D = 2048
N_CORES = 8
LC = L // N_CORES       # 1024 K-rows per core
DC = D // N_CORES       # 256 V/out columns per core
B = 127                 # block rows (127 elements + 1 carry row = 128 contract)
NB = (L + B - 1) // B   # 65 blocks, last has 64 rows
LAST = L - B * (NB - 1)  # 64

_CACHE = {}


def body(nc, tc, K_ap, V_ap, q_ap, out_ap):
    import concourse.mybir as mybir
    from concourse.dve_ops import TENSOR_TENSOR_REDUCE as TTR

    F32 = mybir.dt.float32
    AF = mybir.ActivationFunctionType
    ALU = mybir.AluOpType
    NEG = -1e30

    with (
        tc.tile_pool(name="dram", bufs=1, space="DRAM") as dram,
        tc.tile_pool(name="constp", bufs=1) as constp,
        tc.tile_pool(name="kp", bufs=4) as kp,
        tc.tile_pool(name="prodp", bufs=2) as prodp,
        tc.tile_pool(name="apool", bufs=6) as apool,
        tc.tile_pool(name="ampool", bufs=6) as ampool,
        tc.tile_pool(name="opool", bufs=6) as opool,
        tc.tile_pool(name="rpool", bufs=4) as rpool,
        tc.tile_pool(name="cpool", bufs=6) as cpool,
        tc.tile_pool(name="smallp", bufs=1) as smallp,
        tc.tile_pool(name="pst", bufs=1, space="PSUM") as pstp,
        tc.tile_pool(name="psb", bufs=2, space="PSUM") as psbp,
        tc.tile_pool(name="pso", bufs=4, space="PSUM") as psop,
    ):
        # ----- constants
        ones1 = constp.tile([1, 128], F32)
        nc.vector.memset(ones1[:], 1.0)
        ones128 = constp.tile([128, 128], F32)
        nc.gpsimd.memset(ones128[:], 1.0)
        ones65 = constp.tile([65, 128], F32)
        nc.gpsimd.memset(ones65[:], 1.0)
        ident = constp.tile([128, 128], F32)
        nc.gpsimd.affine_select(
            ident[:], ones128[:], pattern=[[-1, 128]], base=0,
            channel_multiplier=1, compare_op=ALU.is_equal, fill=0.0)

        # ----- stage 0: q broadcast to [128, D] via a stride-0 DMA (reads
        # q 128 times, ~1MB -- cheaper than warming up the PE for it)
        q_bc = constp.tile([128, D], F32)
        nc.sync.dma_start(
            q_bc[:],
            q_ap.rearrange("(a d) -> a d", a=1).partition_broadcast(128))

        # ----- stage 1: s_part[p, t] = <K[8p + t, :], q>
        s_part = smallp.tile([128, 8], F32)
        K_v = K_ap.rearrange("(p t) d -> t p d", t=8)
        for t in range(8):
            kt = kp.tile([128, D], F32, tag="kt")
            nc.sync.dma_start(kt[:], K_v[t])
            prod = prodp.tile([128, D], F32, tag="prod")
            nc.vector._custom_dve(
                TTR, out=prod[:], in0=kt[:], in1=q_bc[:], s0=0.0, s1=1.0,
                accum_out=s_part[:, t:t + 1])

        # ----- collective: AllGather s (each core contributes its 1024)
        b_in = dram.tile([LC], F32)
        b_out = dram.tile([L], F32)
        nc.sync.dma_start(b_in[:].rearrange("(p t) -> p t", t=8), s_part[:])
        nc.gpsimd.collective_compute(
            "AllGather", ALU.bypass,
            replica_groups=[list(range(N_CORES))],
            ins=[b_in.opt()], outs=[b_out.opt()])

        # ----- stage 2: cummax M over s[0:8192]
        s_cm = smallp.tile([128, 64], F32)
        nc.sync.dma_start(s_cm[:], b_out[:].rearrange("(p j) -> p j", p=128))
        Mi = smallp.tile([128, 64], F32)
        nc.vector.tensor_tensor_scan(
            Mi[:], s_cm[:], s_cm[:], initial=NEG,
            op0=ALU.max, op1=ALU.bypass)
        # chunk-end maxes -> [1, 128] -> exclusive cummax -> back to col
        pst1 = pstp.tile([1, 128], F32, tag="pst1")
        nc.tensor.matmul(pst1[:], lhsT=Mi[:, 63:64], rhs=ident[:],
                         start=True, stop=True)
        erow = smallp.tile([1, 128], F32)
        nc.vector.tensor_copy(erow[:], pst1[:])
        frow = smallp.tile([1, 128], F32)
        nc.vector.tensor_tensor_scan(
            frow[:], erow[:], erow[:], initial=NEG,
            op0=ALU.max, op1=ALU.bypass)
        fex = smallp.tile([1, 128], F32)
        nc.vector.memset(fex[0:1, 0:1], NEG)
        nc.vector.tensor_copy(fex[0:1, 1:128], frow[0:1, 0:127])
        pst2 = pstp.tile([128, 1], F32, tag="pst2")
        nc.tensor.matmul(pst2[:], lhsT=fex[:], rhs=ones1[0:1, 0:1],
                         start=True, stop=True)
        fcol = smallp.tile([128, 1], F32)
        nc.vector.tensor_copy(fcol[:], pst2[:])
        # negM[p, j] = -max(Mi[p, j], fcol[p]) = -M[64p + j]
        negM = smallp.tile([128, 64], F32)
        nc.vector.tensor_scalar(negM[:], Mi[:], fcol[:], -1.0,
                                op0=ALU.max, op1=ALU.mult)

        # ----- two-row operand tiles for the per-block build matmul.
        # Compute engines may only START at partition 0/32/64/96, but reads
        # of [0:2] are fine and DMA is exempt, so row 1 is written via DMA.
        # se2: row 0 = s_ext (carry prefix-max at 128b, block s after),
        #      row 1 = ones (DMA'd from ones128 -- single-partition memsets
        #      of 8320 elements are ~8.7us, a flat DMA is cheap)
        se2 = smallp.tile([2, NB * 128], F32)
        nc.vector.memset(se2[0:1, 0:1], NEG)
        nc.vector.memset(se2[0:1, 8192 + 1 + LAST:], 0.0)
        nc.sync.dma_start(
            se2[1:2, :].rearrange("a (p j) -> a p j", j=128), ones65[:])
        nc.sync.dma_start(
            se2[0:1, 0:8192].rearrange("a (b k) -> a b k", k=128)[:, :, 1:128],
            b_out[0:64 * B].rearrange("(b k) -> b k", k=B))
        nc.sync.dma_start(
            se2[0:1, 64 * 128 + 1:64 * 128 + 1 + LAST],
            b_out[64 * B:L].rearrange("(a k) -> a k", a=1))
        # rb2: row 0 = ones, row 1 = -M flat; pad zeros after L
        rb2 = smallp.tile([2, NB * 128], F32)
        nc.sync.dma_start(
            rb2[0:1, :].rearrange("a (p j) -> a p j", j=128), ones65[:])
        nc.sync.dma_start(rb2[1:2, 0:L].rearrange("a (p j) -> a p j", j=64),
                          negM[:])
        # pad region of row 1: zero via DMA from a zeroed SBUF scratch
        zpad = smallp.tile([1, NB * 128 - L], F32)
        nc.vector.memset(zpad[:], 0.0)
        nc.sync.dma_start(rb2[1:2, L:], zpad[:])
        # single-partition -M copy for DVE reads (partition-0 legal)
        negm_row = smallp.tile([1, NB * 128], F32)
        nc.sync.dma_start(negm_row[0:1, 0:L].rearrange("a (p j) -> a p j", j=64),
                          negM[:])
        # boundary prefix-max values: se2[0, 128b] = M[127b - 1], b>=1
        nc.vector.tensor_scalar_mul(
            se2[0:1, :].rearrange("a (b k) -> a b k", k=128)[:, 1:NB, 0:1],
            negm_row[0:1, 0:NB * B].rearrange("a (b k) -> a b k", k=B)[:, 0:NB - 1, B - 1:B],
            -1.0)
        # e_row[b] = exp(M[127b - 1] - M[127(b+1) - 1]): decay factor of
        # the block-boundary carry recursion; e_row[0] = 1 (times C_0 = 0)
        e_pre = smallp.tile([1, 64], F32)
        nc.vector.tensor_tensor(
            e_pre[0:1, 1:64],
            negm_row[0:1, 253:253 + 63 * B].rearrange(
                "a (b k) -> a b k", k=B)[:, :, 0:1],
            negm_row[0:1, 126:126 + 63 * B].rearrange(
                "a (b k) -> a b k", k=B)[:, :, 0:1],
            ALU.subtract)
        e_row = smallp.tile([1, 64], F32)
        nc.vector.memset(e_row[0:1, 0:1], 1.0)
        nc.scalar.activation(e_row[0:1, 1:64], e_pre[0:1, 1:64], AF.Exp)

        # ----- stage 3: main block loop.
        # All main matmuls are carry-free (rhs row 0 = zeros); pso row 0
        # is the block-local tail T_b (via the rotated a_m col 0). The
        # carry recursion C_{b+1} = e_b*C_b + T_b runs entirely on the
        # vector engine (one scalar_tensor_tensor per block, reading T_b
        # straight from PSUM row 0), and each block b>=1 adds its full
        # prefix by one rank-1 matmul: weights = a_m row 0 (which already
        # holds exp(M[127b-1] - M[t])), rhs = C_b.
        NRHS = 8
        rhs_ring = []
        for i in range(NRHS):
            rt = constp.tile([128, DC + 1], F32, tag=f"rhs{i}", name=f"rhs{i}")
            nc.vector.memset(rt[0:128, DC:DC + 1], 1.0)
            nc.vector.memset(rt[0:1, 0:DC + 1], 0.0)
            rhs_ring.append(rt)

        def load_rhs(b):
            rows = B if b < NB - 1 else LAST
            rhs = rhs_ring[b % NRHS]
            if rows < B:
                # last block: zero partitions the V DMA won't cover (their
                # weights are masked to 0, but reads must be defined)
                nc.gpsimd.memset(rhs[64:128, 0:DC], 0.0)
            nc.sync.dma_start(rhs[1:1 + rows, 0:DC],
                              V_ap[B * b:B * b + rows, :])
            return rhs

        C_cur = None
        rhs_t = load_rhs(0)
        for b in range(NB):
            rows = B if b < NB - 1 else LAST
            t0 = B * b
            # S1_ext[p, j] = s_ext[p] - M[t0 + j] in one [2,128]x[2,rows]
            psb = psbp.tile([128, B], F32, tag="psb")
            nc.tensor.matmul(psb[:, 0:rows],
                             lhsT=se2[:, 128 * b:128 * b + 128],
                             rhs=rb2[:, t0:t0 + rows],
                             start=True, stop=True)
            a_sb = apool.tile([128, B], F32, tag="a_sb")
            nc.scalar.activation(a_sb[:, 0:rows], psb[:, 0:rows], AF.Exp)
            # a_m columns rotated by one: col 0 = unmasked last column, so
            # output row 0 = the block-local tail T_b on partition 0
            a_m = ampool.tile([128, B + 1], F32, tag="a_m")
            nc.gpsimd.affine_select(
                a_m[:, 1:1 + rows], a_sb[:, 0:rows], pattern=[[1, rows]],
                base=1, channel_multiplier=-1, compare_op=ALU.is_ge,
                fill=0.0)
            nc.gpsimd.tensor_copy(a_m[:, 0:1], a_sb[:, rows - 1:rows])
            pso = psop.tile([B + 1, DC + 1], F32, tag="pso")
            nc.tensor.matmul(pso[0:1 + rows, :], lhsT=a_m[:, 0:1 + rows],
                             rhs=rhs_t[:], start=True, stop=True)
            if b + 1 < NB:
                next_rhs = load_rhs(b + 1)
            if b == 0:
                C_cur = cpool.tile([1, DC + 1], F32, tag="c")
                nc.vector.tensor_copy(C_cur[:], pso[0:1, 0:DC + 1])
            elif b + 1 < NB:
                C_next = cpool.tile([1, DC + 1], F32, tag="c")
                nc.vector.scalar_tensor_tensor(
                    C_next[:], C_cur[:], e_row[0:1, b:b + 1],
                    pso[0:1, 0:DC + 1], op0=ALU.mult, op1=ALU.add)
            if b > 0:
                # add the missing global prefix: rank-1 accumulate with the
                # carry pseudo-row weights already present in a_m row 0
                nc.tensor.matmul(pso[0:1 + rows, :],
                                 lhsT=a_m[0:1, 0:1 + rows],
                                 rhs=C_cur[:], start=False, stop=True,
                                 skip_group_check=True)
                if b + 1 < NB:
                    C_cur = C_next
            r_t = rpool.tile([B + 1, 1], F32, tag="r")
            nc.vector.reciprocal(r_t[0:1 + rows, :],
                                 pso[0:1 + rows, DC:DC + 1])
            o_sb = opool.tile([B + 1, DC], F32, tag="o")
            nc.scalar.activation(o_sb[0:1 + rows, :], pso[0:1 + rows, 0:DC],
                                 AF.Copy, bias=0.0, scale=r_t[0:1 + rows, :])
            nc.sync.dma_start(out_ap[t0:t0 + rows, :], o_sb[1:1 + rows, :])
            if b + 1 < NB:
                rhs_t = next_rhs


def _build():
    import concourse.bacc as bacc
    import concourse.tile as tile
    import concourse.mybir as mybir

    F32 = mybir.dt.float32
    nc = bacc.Bacc("TRN2", target_bir_lowering=False, debug=False,
                   enable_asserts=True, num_devices=N_CORES)
    K_in = nc.dram_tensor("K", [LC, D], F32, kind="ExternalInput")
    V_in = nc.dram_tensor("V", [L, DC], F32, kind="ExternalInput")
    q_in = nc.dram_tensor("q", [D], F32, kind="ExternalInput")
    o_out = nc.dram_tensor("out", [L, DC], F32, kind="ExternalOutput")
    with tile.TileContext(nc) as tc:
        body(nc, tc, K_in.ap(), V_in.ap(), q_in.ap(), o_out.ap())
    nc.compile()
    return nc


def _get_nc():
    if "nc" not in _CACHE:
        _CACHE["nc"] = _build()
    return _CACHE["nc"]


def shard_inputs(K, V, q):
    K = np.ascontiguousarray(K, dtype=np.float32)
    V = np.ascontiguousarray(V, dtype=np.float32)
    q = np.ascontiguousarray(q, dtype=np.float32)
    return [
        {
            "K": np.ascontiguousarray(K[LC * c:LC * (c + 1), :]),
            "V": np.ascontiguousarray(V[:, DC * c:DC * (c + 1)]),
            "q": q,
        }
        for c in range(N_CORES)
    ]


def kernel(K, V, q):
    from concourse import bass_utils

    nc = _get_nc()
    in_maps = shard_inputs(K, V, q)
    res = bass_utils.run_bass_kernel_spmd(
        nc, in_maps, core_ids=list(range(N_CORES)))
    _CACHE["last_results"] = res
    return np.concatenate(
        [res.results[c]["out"] for c in range(N_CORES)], axis=1)
